# revision 6
# baseline (speedup 1.0000x reference)
"""Trainium2 kernel for nn_Graph_41609643163904.

The reference op is a sequential per-cell scatter sweep over a 48x48 grid
(x outer, y inner): read center v, zero it, add v*W[y,x] to the 5x5
neighborhood.  Every step is linear in the grid, so the sweep is a fixed
linear operator M (2304x2304) of the weights; the baseline ran the full
block-banded M as 188 dense 128x128 matmul blocks.

v3 exploits the sweep's sequentially-semiseparable (SSS) structure: all
influence crossing grid-column x flows through the 88-dim interface
s(x) = [v(x); v(x-1)] (v = per-cell fired values, 44 per column).  In
x-major layout, each 128-row output tile j (covering grid columns
jc_lo..jc_hi) decomposes EXACTLY as

    out_j = sum_k  nearblk[j,k] @ g0_ktile[k]   (k over cols >= jc_lo: 2-3 tiles)
          + U_j @ s(jc_lo - 1)                  (everything to the left)

and the 16 needed states are propagated by a 16-step chain
    s_i = W_i.T @ [s_{i-1}; g0_interior(cols between cuts)]
with K<=128 per matmul.  Total: 47 near + 16 far + 33 chain lhsT blocks
= 95 logical matmuls vs the baseline's 188 dense blocks.  All operands
bf16 (tolerance is 2e-2; bf16 end-to-end lands ~5e-3), which also halves
DMA.  Data-parallel over the 8192-sample batch across 8 cores, no comm.

Device flow per core: x k-tiles + operator stream + interior-column
gathers (SBUF->SBUF) feed a single PE stream: near(j0), near(j1), then
[chain_i -> out(j=i+2)] interleaved so state copies hide under the near
matmuls of the following output tile.
"""

import os

import numpy as np

SIZE = 48
D = 2
KS = 5
N = SIZE * SIZE          # 2304
B = 8192
NCORES = 8
BS = B // NCORES         # 1024 samples per core
P = 128
NT = N // P              # 18 tiles of 128
MW = 512                 # matmul moving-dim (PSUM bank)
NM = BS // MW            # 2 halves


# ---------------------------------------------------------------- plan ----

def _plan():
    js = []
    for j in range(NT):
        r0 = P * j
        jc_lo, jc_hi = r0 // SIZE, (r0 + P - 1) // SIZE
        ncol_lo = jc_lo if j >= 2 else 0
        ncol_hi = min(jc_hi + 2, SIZE - 1)
        kt_lo = (SIZE * ncol_lo) // P
        kt_hi = (SIZE * (ncol_hi + 1) - 1) // P
        js.append(dict(j=j, jc_lo=jc_lo, jc_hi=jc_hi, ncol_lo=ncol_lo,
                       kts=list(range(kt_lo, kt_hi + 1)),
                       cut=jc_lo - 1 if j >= 2 else None))
    cuts = sorted({d["cut"] for d in js if d["cut"] is not None})
    return js, cuts


_JS, _XS = _plan()
_NSTEP = len(_XS)  # 16 chain steps (step 0 = init)


def _step_cols(i):
    """grid columns consumed by chain step i"""
    c0 = 2 if i == 0 else _XS[i - 1] + 1
    return c0, _XS[i]


def _step_k(i):
    c0, c1 = _step_cols(i)
    ng = 44 * (c1 - c0 + 1)            # g0 interior rows
    return ng if i == 0 else 88 + ng   # rhs K (state + g0)


def _gather_segs(i):
    """DMA segments filling chain step i's rhs g0 rows from resident x
    k-tiles.  Returns (dst: 'a'|'b', dst_p0, src_kt, src_p0, n)."""
    c0, c1 = _step_cols(i)
    a_rows = 128 if i == 0 else 40  # g0 rows living in tile_a (after state)
    a_base = 0 if i == 0 else 88
    segs = []
    r = 0  # stack row index
    for c in range(c0, c1 + 1):
        g = SIZE * c + 2  # global x-major row of col c's first interior row
        left = 44
        while left:
            kt = g // P
            n = min(left, P * (kt + 1) - g)
            # split across the a/b boundary
            run = 0
            while run < n:
                if r + run < a_rows:
                    m = min(n - run, a_rows - (r + run))
                    segs.append(("a", a_base + r + run, kt, g - P * kt + run, m))
                else:
                    m = n - run
                    segs.append(("b", r + run - a_rows, kt, g - P * kt + run, m))
                run += m
            g += n
            r += n
            left -= n
    return segs


# ------------------------------------------------------- host operators ----

def _build_M_V(weights):
    """Composed operator M (N,N) and firing-value gradient rows V (1936,N),
    fp64, in the original y-major flattening."""
    M = np.eye(N, dtype=np.float64)
    V = np.zeros((44 * 44, N), dtype=np.float64)
    w = weights.astype(np.float64)
    for x in range(D, SIZE - D):
        for y in range(D, SIZE - D):
            c = y * SIZE + x
            wc = w[y, x]
            rc = M[c].copy()
            V[(x - D) * 44 + (y - D)] = rc
            for dy in range(-D, D + 1):
                r0 = c + dy * SIZE - D
                wrow = wc[dy + D]
                if dy == 0:
                    M[r0:r0 + D] += np.outer(wrow[:D], rc)
                    M[r0 + D + 1:r0 + KS] += np.outer(wrow[D + 1:], rc)
                else:
                    M[r0:r0 + KS] += np.outer(wrow, rc)
            M[c] = wc[D, D] * rc
    return M, V


def _xmajor_idx():
    n = np.arange(N)
    return (n % SIZE) * SIZE + n // SIZE


def _srows(X):
    return np.concatenate([(X - 2) * 44 + np.arange(44),
                           (X - 3) * 44 + np.arange(44)])


def _int_cols(c0, c1):
    return np.concatenate([SIZE * c + 2 + np.arange(44)
                           for c in range(c0, c1 + 1)])


def _build_operators(weights):
    M, V = _build_M_V(weights)
    idx = _xmajor_idx()
    Mx = M[np.ix_(idx, idx)]
    Vx = V[:, idx]
    ops = {}
    for d in _JS:
        j = d["j"]
        jr = slice(P * j, P * j + P)
        e_lo = SIZE * d["ncol_lo"]
        for kt in d["kts"]:
            blk = Mx[jr, P * kt:P * kt + P].copy()
            cols = np.arange(P * kt, P * kt + P)
            blk[:, cols < e_lo] = 0.0
            ops[("near", j, kt)] = blk.T        # lhsT (K=128, M=128)
        if d["cut"] is not None:
            X = d["cut"]
            sf = Vx[_srows(X), :SIZE * (X + 1)]
            F = Mx[jr, :SIZE * d["jc_lo"]]
            U, _, _, _ = np.linalg.lstsq(sf.T, F.T, rcond=None)
            ops[("far", j)] = U                 # lhsT (K=88, M=128)
    for i in range(_NSTEP):
        c0, c1 = _step_cols(i)
        X = _XS[i]
        Binj = Vx[np.ix_(_srows(X), _int_cols(c0, c1))]
        if i == 0:
            W = Binj.T
        else:
            Xp = _XS[i - 1]
            sf_p = Vx[_srows(Xp), :SIZE * (Xp + 1)]
            tgt = Vx[_srows(X), :SIZE * (Xp + 1)]
            T, _, _, _ = np.linalg.lstsq(sf_p.T, tgt.T, rcond=None)
            W = np.vstack([T, Binj.T])
        ops[("chain", i)] = W                   # lhsT (K_i, M=88)
    return ops


# ----------------------------------------------------- operator packing ----

def _pack_layout():
    """Column ranges in the packed wt tensor, in PE consumption order."""
    off = 0
    lay = {}

    def put(key, cols):
        nonlocal off
        lay[key] = (off, cols)
        off += cols

    for kt in _JS[0]["kts"]:
        put(("near", 0, kt), P)
    for kt in _JS[1]["kts"]:
        put(("near", 1, kt), P)
    for i in range(_NSTEP):
        put(("chain_a", i), 88)
        put(("chain_b", i), 88)
        j = i + 2
        for kt in _JS[j]["kts"]:
            put(("near", j, kt), P)
        put(("far", j), P)
    return lay, off


_LAY, _TOTC = _pack_layout()


def _pack_ops(ops):
    wt = np.zeros((P, _TOTC), dtype=np.float32)
    for d in _JS:
        j = d["j"]
        for kt in d["kts"]:
            o, c = _LAY[("near", j, kt)]
            wt[:, o:o + c] = ops[("near", j, kt)]
        if d["cut"] is not None:
            o, c = _LAY[("far", j)]
            wt[:88, o:o + P] = ops[("far", j)]
    for i in range(_NSTEP):
        W = ops[("chain", i)]
        Ktot = W.shape[0]
        o, _ = _LAY[("chain_a", i)]
        wt[:min(Ktot, P), o:o + 88] = W[:P]
        if Ktot > P:
            o, _ = _LAY[("chain_b", i)]
            wt[:Ktot - P, o:o + 88] = W[P:]
    return wt


# fetch groups: (group key list of layout keys) in consumption order
def _fetch_groups():
    gs = []
    gs.append(("near0", [("near", 0, kt) for kt in _JS[0]["kts"]]))
    gs.append(("near1", [("near", 1, kt) for kt in _JS[1]["kts"]]))
    for i in range(_NSTEP):
        gs.append((f"ch{i}", [("chain_a", i), ("chain_b", i)]))
        j = i + 2
        gs.append((f"out{j}",
                   [("near", j, kt) for kt in _JS[j]["kts"]] + [("far", j)]))
    return gs


_FETCH = _fetch_groups()
_WMAX = max(sum(_LAY[k][1] for k in keys) for _, keys in _FETCH)


# ------------------------------------------------------------- device ----

def _build_device_kernel():
    import concourse.mybir as mybir
    from concourse import bacc
    from concourse.tile import TileContext

    f32 = mybir.dt.float32
    bf16 = mybir.dt.bfloat16

    nc = bacc.Bacc()
    xT = nc.dram_tensor("xT", [N, BS], bf16, kind="ExternalInput")
    wt = nc.dram_tensor("wt", [P, _TOTC], bf16, kind="ExternalInput")
    outT = nc.dram_tensor("outT", [N, BS], bf16, kind="ExternalOutput")

    with TileContext(nc) as tc:
        with (
            tc.tile_pool(name="xpool", bufs=1) as xpool,
            tc.tile_pool(name="apool", bufs=1) as apool,
            tc.tile_pool(name="bpool", bufs=1) as bpool,
            tc.tile_pool(name="wpool", bufs=6) as wpool,
            tc.tile_pool(name="opool", bufs=3) as opool,
            tc.tile_pool(name="pso", bufs=2, space="PSUM") as pso,
            tc.tile_pool(name="pss", bufs=2, space="PSUM") as pss,
        )        :
            xtiles = {}

            def issue_xk(t):
                if t in xtiles or t >= NT:
                    return
                xk = xpool.tile([P, BS], bf16, tag=f"x{t}", name=f"x{t}")
                nc.scalar.dma_start(out=xk[:], in_=xT[P * t:P * t + P, :])
                xtiles[t] = xk

            # chain rhs tiles (state + g0 interior stacks)
            ta = [apool.tile([P, BS], bf16, tag=f"a{i}", name=f"a{i}")
                  for i in range(_NSTEP)]
            tb = [bpool.tile([92, BS], bf16, tag=f"b{i}", name=f"b{i}")
                  for i in range(_NSTEP)]
            s_last = apool.tile([88, BS], bf16, tag="slast", name="slast")

            def issue_gathers(i):
                for dst, dp, kt, sp, n in _gather_segs(i):
                    tile = ta[i] if dst == "a" else tb[i]
                    nc.scalar.dma_start(out=tile[dp:dp + n, :],
                                        in_=xtiles[kt][sp:sp + n, :])

            wslot = {}

            def fetch_w(gkey):
                keys = dict(_FETCH)[gkey]
                cols = sum(_LAY[k][1] for k in keys)
                o0 = _LAY[keys[0]][0]
                wtile = wpool.tile([P, _WMAX], bf16, tag="w", name=f"w_{gkey}")
                nc.sync.dma_start(out=wtile[:, :cols],
                                  in_=wt[:, o0:o0 + cols])
                for k in keys:
                    wslot[k] = (wtile, _LAY[k][0] - o0)
                return wtile

            def w_ap(key, kk):
                wtile, o = wslot[key]
                m = 88 if key[0].startswith("chain") else P
                return wtile[0:kk, o:o + m]

            def out_group(j):
                d = _JS[j]
                items = [("near", kt) for kt in d["kts"]]
                if d["cut"] is not None:
                    items.append(("far", None))
                ps = [pso.tile([P, MW], f32, tag=f"o{m}", name=f"ps{j}_{m}")
                      for m in range(NM)]
                for it, (kind, kt) in enumerate(items):
                    first, last = it == 0, it == len(items) - 1
                    for m in range(NM):
                        if kind == "near":
                            lhsT = w_ap(("near", j, kt), P)
                            rhs = xtiles[kt][:, m * MW:(m + 1) * MW]
                        else:
                            lhsT = w_ap(("far", j), 88)
                            st = ta[j - 1] if j - 1 < _NSTEP else s_last
                            rhs = st[0:88, m * MW:(m + 1) * MW]
                        nc.tensor.matmul(ps[m][:], lhsT=lhsT, rhs=rhs,
                                         start=first, stop=last)
                oc = opool.tile([P, BS], bf16, tag="o", name=f"oc{j}")
                for m in range(NM):
                    eng = nc.vector.tensor_copy if (j + m) % 2 == 0 \
                        else nc.scalar.copy
                    eng(oc[:, m * MW:(m + 1) * MW], ps[m][:])
                nc.gpsimd.dma_start(out=outT[P * j:P * j + P, :], in_=oc[:])

            def chain_step(i):
                kk = _step_k(i)
                ka = min(kk, P)
                kb = kk - ka
                ps = [pss.tile([88, MW], f32, tag=f"s{m}", name=f"pss{i}_{m}")
                      for m in range(NM)]
                items = [("chain_a", ka, ta[i])]
                if kb:
                    items.append(("chain_b", kb, tb[i]))
                for it, (wk, kdim, rt) in enumerate(items):
                    first, last = it == 0, it == len(items) - 1
                    for m in range(NM):
                        nc.tensor.matmul(
                            ps[m][:], lhsT=w_ap((wk, i), kdim),
                            rhs=rt[0:kdim, m * MW:(m + 1) * MW],
                            start=first, stop=last)
                dst = ta[i + 1] if i + 1 < _NSTEP else s_last
                for m in range(NM):
                    nc.scalar.copy(dst[0:88, m * MW:(m + 1) * MW], ps[m][:])

            # ---------------- emission ----------------
            issue_xk(0)
            issue_xk(1)
            issue_xk(2)
            fetch_w("near0")
            out_group(0)
            fetch_w("near1")
            out_group(1)
            for i in range(_NSTEP):
                issue_gathers(i)   # needs xk <= i+1 (already issued)
                issue_xk(i + 3)    # out_group(i+2) consumes k-tiles <= i+3
                fetch_w(f"ch{i}")
                chain_step(i)
                fetch_w(f"out{i + 2}")
                out_group(i + 2)

    if not nc.is_finalized():
        nc.finalize()
    return nc


# -------------------------------------------------------------- driver ----

def kernel(inputs: np.ndarray, weights: np.ndarray) -> np.ndarray:
    import ml_dtypes
    from concourse.bass_utils import run_bass_kernel_spmd

    inputs = np.ascontiguousarray(inputs, dtype=np.float32)
    weights = np.ascontiguousarray(weights, dtype=np.float32)

    ops = _build_operators(weights)
    wt_packed = np.ascontiguousarray(_pack_ops(ops)).astype(ml_dtypes.bfloat16)

    # x-major per-sample flatten, then transpose so grid index leads
    xP = inputs.reshape(B, SIZE, SIZE).transpose(0, 2, 1).reshape(B, N)

    nc = _build_device_kernel()
    in_maps = [
        {
            "xT": np.ascontiguousarray(xP[c * BS:(c + 1) * BS].T)
            .astype(ml_dtypes.bfloat16),
            "wt": wt_packed,
        }
        for c in range(NCORES)
    ]
    trace = bool(int(os.environ.get("KERNEL_TRACE", "0")))
    res = run_bass_kernel_spmd(
        nc, in_maps, core_ids=list(range(NCORES)), trace=trace
    )
    if trace and res.exec_time_ns is not None:
        print(f"HW exec time: {res.exec_time_ns} ns")
        if res.instructions_and_trace is not None:
            print(f"trace: {res.instructions_and_trace[1]}")

    outP = np.concatenate(
        [res.results[c]["outT"].astype(np.float32).T for c in range(NCORES)],
        axis=0,
    )
    return np.ascontiguousarray(
        outP.reshape(B, SIZE, SIZE).transpose(0, 2, 1).reshape(B, N)
    )


# revision 10
# speedup vs baseline: 1.6501x; 1.6501x over previous
"""Trainium2 kernel for nn_Graph_41609643163904.

The reference op is a sequential per-cell scatter sweep over a 48x48 grid
(x outer, y inner): read center v, zero it, add v*W[y,x] to the 5x5
neighborhood.  Every step is linear in the grid, so the sweep is a fixed
linear operator M (2304x2304) of the weights; the baseline ran the full
block-banded M as 188 dense 128x128 matmul blocks.

v3 exploits the sweep's sequentially-semiseparable (SSS) structure: all
influence crossing grid-column x flows through the 88-dim interface
s(x) = [v(x); v(x-1)] (v = per-cell fired values, 44 per column).  In
x-major layout, each 128-row output tile j (covering grid columns
jc_lo..jc_hi) decomposes EXACTLY as

    out_j = sum_k  nearblk[j,k] @ g0_ktile[k]   (k over cols >= jc_lo: 2-3 tiles)
          + U_j @ s(jc_lo - 1)                  (everything to the left)

and the 16 needed states are propagated by a 16-step chain
    s_i = W_i.T @ [s_{i-1}; g0_interior(cols between cuts)]
with K<=128 per matmul.  Total: 47 near + 16 far + 33 chain lhsT blocks
= 95 logical matmuls vs the baseline's 188 dense blocks.  All operands
bf16 (tolerance is 2e-2; bf16 end-to-end lands ~5e-3), which also halves
DMA.  Data-parallel over the 8192-sample batch across 8 cores, no comm.

Device flow per core: x k-tiles + operator stream + interior-column
gathers (SBUF->SBUF) feed a single PE stream: near(j0), near(j1), then
[chain_i -> out(j=i+2)] interleaved so state copies hide under the near
matmuls of the following output tile.
"""

import os

import numpy as np

SIZE = 48
D = 2
KS = 5
N = SIZE * SIZE          # 2304
B = 8192
NCORES = 8
BS = B // NCORES         # 1024 samples per core
P = 128
NT = N // P              # 18 tiles of 128
MW = 512                 # matmul moving-dim (PSUM bank)
NM = BS // MW            # 2 halves


# ---------------------------------------------------------------- plan ----

def _plan():
    js = []
    for j in range(NT):
        r0 = P * j
        jc_lo, jc_hi = r0 // SIZE, (r0 + P - 1) // SIZE
        ncol_lo = jc_lo if j >= 2 else 0
        ncol_hi = min(jc_hi + 2, SIZE - 1)
        kt_lo = (SIZE * ncol_lo) // P
        kt_hi = (SIZE * (ncol_hi + 1) - 1) // P
        js.append(dict(j=j, jc_lo=jc_lo, jc_hi=jc_hi, ncol_lo=ncol_lo,
                       kts=list(range(kt_lo, kt_hi + 1)),
                       cut=jc_lo - 1 if j >= 2 else None))
    cuts = sorted({d["cut"] for d in js if d["cut"] is not None})
    return js, cuts


_JS, _XS = _plan()
_NSTEP = len(_XS)  # 16 chain steps (step 0 = init)


def _step_cols(i):
    """grid columns consumed by chain step i"""
    c0 = 2 if i == 0 else _XS[i - 1] + 1
    return c0, _XS[i]


def _step_k(i):
    c0, c1 = _step_cols(i)
    ng = SIZE * (c1 - c0 + 1)          # FULL g0 columns (boundary rows get
    return ng if i == 0 else 88 + ng   # zero operator coeffs)


def _gather_segs(i):
    """DMA segments filling chain step i's rhs g0 rows (a contiguous
    x-major row range) from resident x k-tiles.
    Returns (dst: 'a'|'b', dst_p0, src_kt, src_p0, n)."""
    c0, c1 = _step_cols(i)
    a_rows = 128 if i == 0 else 40  # g0 rows living in tile_a (after state)
    a_base = 0 if i == 0 else 88
    g_lo, g_hi = SIZE * c0, SIZE * (c1 + 1)  # global row range
    segs = []
    r = 0
    g = g_lo
    while g < g_hi:
        kt = g // P
        n = min(g_hi - g, P * (kt + 1) - g)
        run = 0
        while run < n:
            if r + run < a_rows:
                m = min(n - run, a_rows - (r + run))
                segs.append(("a", a_base + r + run, kt, g - P * kt + run, m))
            else:
                m = n - run
                segs.append(("b", r + run - a_rows, kt, g - P * kt + run, m))
            run += m
        g += n
        r += n
    return segs


# ------------------------------------------------------- host operators ----

def _build_M_V(weights):
    """Composed operator M (N,N) and firing-value gradient rows V (1936,N),
    fp64, in the original y-major flattening."""
    M = np.eye(N, dtype=np.float64)
    V = np.zeros((44 * 44, N), dtype=np.float64)
    w = weights.astype(np.float64)
    for x in range(D, SIZE - D):
        for y in range(D, SIZE - D):
            c = y * SIZE + x
            wc = w[y, x]
            rc = M[c].copy()
            V[(x - D) * 44 + (y - D)] = rc
            for dy in range(-D, D + 1):
                r0 = c + dy * SIZE - D
                wrow = wc[dy + D]
                if dy == 0:
                    M[r0:r0 + D] += np.outer(wrow[:D], rc)
                    M[r0 + D + 1:r0 + KS] += np.outer(wrow[D + 1:], rc)
                else:
                    M[r0:r0 + KS] += np.outer(wrow, rc)
            M[c] = wc[D, D] * rc
    return M, V


def _xmajor_idx():
    n = np.arange(N)
    return (n % SIZE) * SIZE + n // SIZE


def _srows(X):
    return np.concatenate([(X - 2) * 44 + np.arange(44),
                           (X - 3) * 44 + np.arange(44)])


def _int_cols(c0, c1):
    return np.concatenate([SIZE * c + 2 + np.arange(44)
                           for c in range(c0, c1 + 1)])


def _build_operators(weights):
    M, V = _build_M_V(weights)
    idx = _xmajor_idx()
    Mx = M[np.ix_(idx, idx)]
    Vx = V[:, idx]
    ops = {}
    for d in _JS:
        j = d["j"]
        jr = slice(P * j, P * j + P)
        e_lo = SIZE * d["ncol_lo"]
        for kt in d["kts"]:
            blk = Mx[jr, P * kt:P * kt + P].copy()
            cols = np.arange(P * kt, P * kt + P)
            blk[:, cols < e_lo] = 0.0
            ops[("near", j, kt)] = blk.T        # lhsT (K=128, M=128)
        if d["cut"] is not None:
            X = d["cut"]
            sf = Vx[_srows(X), :SIZE * (X + 1)]
            F = Mx[jr, :SIZE * d["jc_lo"]]
            U, _, _, _ = np.linalg.lstsq(sf.T, F.T, rcond=None)
            ops[("far", j)] = U                 # lhsT (K=88, M=128)
    for i in range(_NSTEP):
        c0, c1 = _step_cols(i)
        X = _XS[i]
        Binj = Vx[_srows(X), SIZE * c0:SIZE * (c1 + 1)]  # full columns
        if i == 0:
            W = Binj.T
        else:
            Xp = _XS[i - 1]
            sf_p = Vx[_srows(Xp), :SIZE * (Xp + 1)]
            tgt = Vx[_srows(X), :SIZE * (Xp + 1)]
            T, _, _, _ = np.linalg.lstsq(sf_p.T, tgt.T, rcond=None)
            W = np.vstack([T, Binj.T])
        ops[("chain", i)] = W                   # lhsT (K_i, M=88)
    return ops


# ----------------------------------------------------- operator packing ----

def _pack_layout():
    """Column ranges in the packed wt tensor, in PE consumption order."""
    off = 0
    lay = {}

    def put(key, cols):
        nonlocal off
        lay[key] = (off, cols)
        off += cols

    for kt in _JS[0]["kts"]:
        put(("near", 0, kt), P)
    for kt in _JS[1]["kts"]:
        put(("near", 1, kt), P)
    for i in range(_NSTEP):
        put(("chain_a", i), 88)
        put(("chain_b", i), 88)
        j = i + 2
        for kt in _JS[j]["kts"]:
            put(("near", j, kt), P)
        put(("far", j), P)
    return lay, off


_LAY, _TOTC = _pack_layout()


def _pack_ops(ops):
    wt = np.zeros((P, _TOTC), dtype=np.float32)
    for d in _JS:
        j = d["j"]
        for kt in d["kts"]:
            o, c = _LAY[("near", j, kt)]
            wt[:, o:o + c] = ops[("near", j, kt)]
        if d["cut"] is not None:
            o, c = _LAY[("far", j)]
            wt[:88, o:o + P] = ops[("far", j)]
    for i in range(_NSTEP):
        W = ops[("chain", i)]
        Ktot = W.shape[0]
        o, _ = _LAY[("chain_a", i)]
        wt[:min(Ktot, P), o:o + 88] = W[:P]
        if Ktot > P:
            o, _ = _LAY[("chain_b", i)]
            wt[:Ktot - P, o:o + 88] = W[P:]
    return wt


# fetch groups: (group key list of layout keys) in consumption order
def _fetch_groups():
    gs = []
    gs.append(("near0", [("near", 0, kt) for kt in _JS[0]["kts"]]))
    gs.append(("near1", [("near", 1, kt) for kt in _JS[1]["kts"]]))
    for i in range(_NSTEP):
        gs.append((f"ch{i}", [("chain_a", i), ("chain_b", i)]))
        j = i + 2
        gs.append((f"out{j}",
                   [("near", j, kt) for kt in _JS[j]["kts"]] + [("far", j)]))
    return gs


_FETCH = _fetch_groups()
_WMAX = max(sum(_LAY[k][1] for k in keys) for _, keys in _FETCH)


# ------------------------------------------------------------- device ----

def _build_device_kernel():
    import concourse.mybir as mybir
    from concourse import bacc
    from concourse.tile import TileContext

    f32 = mybir.dt.float32
    bf16 = mybir.dt.bfloat16

    nc = bacc.Bacc()
    xT = nc.dram_tensor("xT", [N, BS], bf16, kind="ExternalInput")
    wt = nc.dram_tensor("wt", [P, _TOTC], bf16, kind="ExternalInput")
    outT = nc.dram_tensor("outT", [N, BS], bf16, kind="ExternalOutput")

    with TileContext(nc) as tc:
        with (
            tc.tile_pool(name="xpool", bufs=1) as xpool,
            tc.tile_pool(name="apool", bufs=1) as apool,
            tc.tile_pool(name="bpool", bufs=1) as bpool,
            tc.tile_pool(name="wpool", bufs=6) as wpool,
            tc.tile_pool(name="opool", bufs=3) as opool,
            tc.tile_pool(name="pso", bufs=2, space="PSUM") as pso,
            tc.tile_pool(name="pss", bufs=2, space="PSUM") as pss,
        ):
            # Engine / DMA-ring roles:
            #   sync   (SP HWDGE): x k-tiles + chain-stack gathers
            #   scalar (ACT HWDGE): operator stream + chain state copies
            #   vector (DVE): PSUM->SBUF output copies
            #   gpsimd (SWDGE): output stores
            xtiles = {}

            def issue_xk(t):
                if t in xtiles or t >= NT:
                    return
                xk = xpool.tile([P, BS], bf16, tag=f"x{t}", name=f"x{t}")
                nc.sync.dma_start(out=xk[:], in_=xT[P * t:P * t + P, :])
                xtiles[t] = xk

            # chain rhs tiles (state + full-column g0 stacks)
            ta = [apool.tile([P, BS], bf16, tag=f"a{i}", name=f"a{i}")
                  for i in range(_NSTEP)]
            tb = [bpool.tile([104, BS], bf16, tag=f"b{i}", name=f"b{i}")
                  for i in range(_NSTEP)]
            s_last = apool.tile([88, BS], bf16, tag="slast", name="slast")

            gathered = set()

            def issue_gathers(i):
                if i in gathered or i >= _NSTEP:
                    return
                gathered.add(i)
                for dst, dp, kt, sp, n in _gather_segs(i):
                    tile = ta[i] if dst == "a" else tb[i]
                    nc.sync.dma_start(out=tile[dp:dp + n, :],
                                      in_=xtiles[kt][sp:sp + n, :])

            wslot = {}
            wfetched = set()

            def fetch_w(gkey):
                if gkey in wfetched:
                    return
                wfetched.add(gkey)
                keys = dict(_FETCH)[gkey]
                cols = sum(_LAY[k][1] for k in keys)
                o0 = _LAY[keys[0]][0]
                wtile = wpool.tile([P, _WMAX], bf16, tag="w", name=f"w_{gkey}")
                nc.scalar.dma_start(out=wtile[:, :cols],
                                    in_=wt[:, o0:o0 + cols])
                for k in keys:
                    wslot[k] = (wtile, _LAY[k][0] - o0)

            def w_ap(key, kk):
                wtile, o = wslot[key]
                m = 88 if key[0].startswith("chain") else P
                return wtile[0:kk, o:o + m]

            def out_group(j):
                d = _JS[j]
                items = [("near", kt) for kt in d["kts"]]
                if d["cut"] is not None:
                    items.append(("far", None))
                ps = pso.tile([P, BS], f32, tag="o", name=f"ps{j}")
                for it, (kind, kt) in enumerate(items):
                    first, last = it == 0, it == len(items) - 1
                    for m in range(NM):
                        if kind == "near":
                            lhsT = w_ap(("near", j, kt), P)
                            rhs = xtiles[kt][:, m * MW:(m + 1) * MW]
                        else:
                            lhsT = w_ap(("far", j), 88)
                            st = ta[j - 1] if j - 1 < _NSTEP else s_last
                            rhs = st[0:88, m * MW:(m + 1) * MW]
                        nc.tensor.matmul(ps[:, m * MW:(m + 1) * MW],
                                         lhsT=lhsT, rhs=rhs,
                                         start=first, stop=last)
                oc = opool.tile([P, BS], bf16, tag="o", name=f"oc{j}")
                nc.vector.tensor_copy(oc[:], ps[:])
                nc.gpsimd.dma_start(out=outT[P * j:P * j + P, :], in_=oc[:])

            def chain_step(i):
                kk = _step_k(i)
                ka = min(kk, P)
                kb = kk - ka
                ps = pss.tile([88, BS], f32, tag="s", name=f"pss{i}")
                items = [("chain_a", ka, ta[i])]
                if kb:
                    items.append(("chain_b", kb, tb[i]))
                for it, (wk, kdim, rt) in enumerate(items):
                    first, last = it == 0, it == len(items) - 1
                    for m in range(NM):
                        nc.tensor.matmul(
                            ps[:, m * MW:(m + 1) * MW],
                            lhsT=w_ap((wk, i), kdim),
                            rhs=rt[0:kdim, m * MW:(m + 1) * MW],
                            start=first, stop=last)
                dst = ta[i + 1] if i + 1 < _NSTEP else s_last
                nc.scalar.copy(dst[0:88, :], ps[:])

            # ---------------- emission ----------------
            wqueue = [g for g, _ in _FETCH]  # consumption order
            wptr = 0

            def fetch_more(n):
                nonlocal wptr
                for _ in range(n):
                    if wptr < len(wqueue):
                        fetch_w(wqueue[wptr])
                        wptr += 1

            for t in range(4):
                issue_xk(t)
            issue_gathers(0)
            issue_gathers(1)
            fetch_more(6)
            out_group(0)
            out_group(1)
            for i in range(_NSTEP):
                issue_xk(i + 4)
                issue_gathers(i + 2)
                fetch_more(2)
                chain_step(i)
                out_group(i + 2)

    if not nc.is_finalized():
        nc.finalize()
    return nc


# -------------------------------------------------------------- driver ----

def kernel(inputs: np.ndarray, weights: np.ndarray) -> np.ndarray:
    import ml_dtypes
    from concourse.bass_utils import run_bass_kernel_spmd

    inputs = np.ascontiguousarray(inputs, dtype=np.float32)
    weights = np.ascontiguousarray(weights, dtype=np.float32)

    ops = _build_operators(weights)
    wt_packed = np.ascontiguousarray(_pack_ops(ops)).astype(ml_dtypes.bfloat16)

    # x-major per-sample flatten, then transpose so grid index leads
    xP = inputs.reshape(B, SIZE, SIZE).transpose(0, 2, 1).reshape(B, N)

    nc = _build_device_kernel()
    in_maps = [
        {
            "xT": np.ascontiguousarray(xP[c * BS:(c + 1) * BS].T)
            .astype(ml_dtypes.bfloat16),
            "wt": wt_packed,
        }
        for c in range(NCORES)
    ]
    trace = bool(int(os.environ.get("KERNEL_TRACE", "0")))
    res = run_bass_kernel_spmd(
        nc, in_maps, core_ids=list(range(NCORES)), trace=trace
    )
    if trace and res.exec_time_ns is not None:
        print(f"HW exec time: {res.exec_time_ns} ns")
        if res.instructions_and_trace is not None:
            print(f"trace: {res.instructions_and_trace[1]}")

    outP = np.concatenate(
        [res.results[c]["outT"].astype(np.float32).T for c in range(NCORES)],
        axis=0,
    )
    return np.ascontiguousarray(
        outP.reshape(B, SIZE, SIZE).transpose(0, 2, 1).reshape(B, N)
    )
